# revision 5
# baseline (speedup 1.0000x reference)
"""Trainium2 Bass kernel for nn_ExpandMask (stride 2, padding 2).

Reference op (per batch row, x of length L, fp32 in [0,1)):
  zero-stuff by stride 2 -> conv1d(ones, width 5, 'same') -> (> 0.5)

Mathematically, for i in [0, L):
  out[2i]   = (x[i-1] + x[i] + x[i+1]) > 0.5     (x[-1] = x[L] = 0)
  out[2i+1] = (x[i] + x[i+1]) > 0.5

Sharding: pure data parallel — the batch dim (64 rows) is split across
8 NeuronCores, 8 rows per core; the op is local along L so there is no
communication.

Per-core kernel (fp16 sums; the 2e-2 rel-err gate dwarfs the ~1e-4
flip rate near the 0.5 threshold):
  - Each batch row is one [128 x 2048] fp32 block with halo columns.
  - Convert fp32 -> fp16 (split between GpSimd and ACT so no engine
    owns a full extra pass).
  - DVE does the two adds in fp16 (2x perf mode) and the odd compare
    as tensor_scalar is_gt -> fp16 {0,1} (4x perf mode).
  - ACT does the even compare as one saturated-sigmoid pass -> fp16
    {0,1}: sigmoid(2^40*s3 - 2^40*(0.5+2^-13)) is exactly 0/1 for all
    fp16 grid values (threshold nudged half an ulp so s3 == 0.5 -> 0).
  - PE packs 8 bits/byte: 8 accumulating matmuls per block with
    block-diagonal power-of-two fp16 weights compress the [128 x 2048]
    even+odd bit planes into one dense [128 x 512] fp32 PSUM tile of
    byte values (0..255, exact in fp32).
  - ACT copies PSUM -> u8 SBUF; DMA stores 64KB/block (8x less output
    traffic than unpacked bool planes). The host unpacks bits during
    unsharding.
"""

import sys

import numpy as np

sys.path.insert(0, "/opt/trn_rl_repo")

import concourse.bass as bass  # noqa: E402
from concourse import bacc, mybir  # noqa: E402
from concourse.bass_utils import run_bass_kernel_spmd  # noqa: E402
from concourse.mybir import AluOpType  # noqa: E402
from concourse.tile import TileContext  # noqa: E402

B = 64
L = 262144
NCORES = 8
RPC = B // NCORES  # rows per core = 8
P = 128
W = L // P  # 2048 payload columns per block (one batch row per block)
NK = 8  # pack chunks per block
C = W * 2 // NK  # 512 columns per packed chunk

SCALE = float(2.0**40)
BIAS = -float(2.0**40 * (0.5 + 2.0**-13))

N_CVT_ACT = 1  # blocks whose fp32->fp16 convert runs on ACT (rest GpSimd)

_CACHE = {}


def _pack_weights() -> np.ndarray:
    """[128, 8*128] fp16: W_k = cols [128k, 128k+128); W_k[p, 16k + p//8]
    = 2^(p%8). Chunk k's matmul maps bit (p%8) of partition-group p//8
    into PSUM partition 16k + p//8; the 8 chunks fill disjoint partition
    ranges of one accumulating [128, 512] PSUM tile."""
    wts = np.zeros((P, NK * P), dtype=np.float16)
    for k in range(NK):
        for p in range(P):
            wts[p, P * k + 16 * k + p // 8] = np.float16(2.0 ** (p % 8))
    return wts


def _build():
    if "nc" in _CACHE:
        return _CACHE["nc"]

    nc = bacc.Bacc(
        "TRN2", target_bir_lowering=False, debug=False, num_devices=NCORES
    )
    f32 = mybir.dt.float32
    f16 = mybir.dt.float16
    u8 = mybir.dt.uint8

    x_in = nc.dram_tensor("x", [RPC, L], f32, kind="ExternalInput")
    w_in = nc.dram_tensor("wpack", [P, NK * P], f16, kind="ExternalInput")
    y_out = nc.dram_tensor("y", [RPC, P * C], u8, kind="ExternalOutput")

    with TileContext(nc) as tc:
        with (
            tc.tile_pool(name="consts", bufs=1) as cpool,
            tc.tile_pool(name="pool", bufs=2) as pool,
            tc.tile_pool(name="psum", bufs=3, space="PSUM") as ppool,
        ):
            bias_big = cpool.tile([P, 1], f32)
            nc.vector.memset(bias_big[:], BIAS)
            wt = cpool.tile([P, NK * P], f16)
            nc.sync.dma_start(out=wt[:], in_=w_in[:, :])

            for b in range(RPC):
                base = b * P * W
                X = pool.tile([P, W + 2], f32, tag="X", bufs=3)
                Xh = pool.tile([P, W + 2], f16, tag="Xh", bufs=2)
                s2x = pool.tile([P, W + 1], f16, tag="s2x", bufs=2)
                s3 = pool.tile([P, W], f16, tag="s3", bufs=2)
                od = pool.tile([P, W], f16, tag="od", bufs=2)
                ev = pool.tile([P, W], f16, tag="ev", bufs=2)

                # X[:, j] = x[p*W + j - 1]: left/right halo in cols 0 and
                # W+1 ride along in the flat load from base-1. The three
                # row-boundary cells this gets wrong (X[0,0] reads the
                # previous row's last element, X[127,W+1] the next row's
                # first) stay finite, corrupt only the row's first even
                # bit and last even/odd bits, and the host recomputes
                # those exactly from the fp32 input during unsharding.
                # Only the core-slab ends need special casing: reading
                # outside the x tensor is illegal, and the stale-SBUF
                # cells it would leave could be NaN, which would poison
                # the other 7 bits of their packed byte.
                if b == 0:
                    nc.sync.dma_start(
                        out=X[:, 1 : W + 2],
                        in_=bass.AP(x_in, 0, [[W, P], [1, W + 1]]),
                    )
                    nc.gpsimd.memset(X[:, 0:1], 0.0)
                    nc.sync.dma_start(
                        out=X[1:P, 0:1],
                        in_=bass.AP(x_in, W - 1, [[W, P - 1], [1, 1]]),
                    )
                elif b == RPC - 1:
                    nc.sync.dma_start(
                        out=X[:, 0 : W + 1],
                        in_=bass.AP(x_in, base - 1, [[W, P], [1, W + 1]]),
                    )
                    nc.gpsimd.memset(X[:, W + 1 : W + 2], 0.0)
                    nc.sync.dma_start(
                        out=X[0 : P - 1, W + 1 : W + 2],
                        in_=bass.AP(x_in, base + W, [[W, P - 1], [1, 1]]),
                    )
                else:
                    nc.sync.dma_start(
                        out=X[:, 0 : W + 2],
                        in_=bass.AP(x_in, base - 1, [[W, P], [1, W + 2]]),
                    )

                # fp32 -> fp16
                if b < RPC - N_CVT_ACT:
                    nc.gpsimd.tensor_copy(Xh[:], X[:])
                else:
                    nc.scalar.activation(
                        Xh[:], X[:], mybir.ActivationFunctionType.Copy
                    )

                # s2x[:, j] = x[j-1] + x[j]  (j in 0..W)
                nc.vector.tensor_tensor(
                    s2x[:], Xh[:, 0 : W + 1], Xh[:, 1 : W + 2], AluOpType.add
                )
                # s3[:, i] = s2x[:, i] + x[i+1]  (reference add order)
                nc.vector.tensor_tensor(
                    s3[:], s2x[:, 0:W], Xh[:, 2 : W + 2], AluOpType.add
                )
                # odd bits: (x[i] + x[i+1]) > 0.5 -> {0,1} fp16 (DVE 4x)
                nc.vector.tensor_scalar(
                    od[:], s2x[:, 1 : W + 1], 0.5, None, AluOpType.is_gt
                )
                # even bits: saturated sigmoid -> {0,1} fp16
                nc.scalar.activation(
                    ev[:],
                    s3[:],
                    mybir.ActivationFunctionType.Sigmoid,
                    bias=bias_big[:],
                    scale=SCALE,
                )

                # pack both planes into one [128, 512] PSUM tile of bytes
                ps = ppool.tile([P, C], mybir.dt.float32, tag="ps", bufs=3)
                for k in range(NK):
                    src = ev if k < NK // 2 else od
                    c0 = (k % (NK // 2)) * C
                    nc.tensor.matmul(
                        ps[:],
                        wt[:, P * k : P * (k + 1)],
                        src[:, c0 : c0 + C],
                        start=(k == 0),
                        stop=(k == NK - 1),
                    )
                pk = pool.tile([P, C], u8, tag="pk", bufs=2)
                nc.scalar.activation(
                    pk[:], ps[:], mybir.ActivationFunctionType.Copy
                )
                st = nc.scalar.dma_start(
                    out=bass.AP(y_out, b * P * C, [[C, P], [1, C]]),
                    in_=pk[:],
                )
                try:
                    st.ins.bass_priority = 100
                except AttributeError:
                    st.bass_priority = 100

    nc.compile()
    _CACHE["nc"] = nc
    return nc


def _unpack_core(y: np.ndarray) -> tuple[np.ndarray, np.ndarray]:
    """[RPC, P*C] u8 -> (ev, od) bool [RPC, L].

    Per row: [128, 512] bytes; PSUM partition 16k+j, col n holds bits m
    of plane partitions 8j+m at plane col 512*(k%4)+n (k<4: even plane,
    k>=4: odd plane)."""
    arr = y.reshape(RPC, NK, 16, C)  # [r, k, j, n]
    bits = np.unpackbits(arr[..., None], axis=-1, bitorder="little")
    bits = bits.transpose(0, 1, 2, 4, 3)  # [r, k, j, m, n]
    bits = bits.reshape(RPC, NK, P, C)  # [r, k, p=8j+m, n]
    h = NK // 2
    ev = bits[:, 0:h].transpose(0, 2, 1, 3).reshape(RPC, L)
    od = bits[:, h:NK].transpose(0, 2, 1, 3).reshape(RPC, L)
    return ev.astype(np.bool_), od.astype(np.bool_)


def kernel(x: np.ndarray) -> np.ndarray:
    assert x.shape == (B, 1, L), x.shape
    x = np.ascontiguousarray(np.asarray(x, dtype=np.float32))

    nc = _build()
    wts = _pack_weights()
    in_maps = [
        {
            "x": np.ascontiguousarray(x[c * RPC : (c + 1) * RPC, 0, :]),
            "wpack": wts,
        }
        for c in range(NCORES)
    ]
    res = run_bass_kernel_spmd(nc, in_maps, core_ids=list(range(NCORES)))
    out = np.empty((B, 1, 2 * L), dtype=np.bool_)
    for c, r in enumerate(res.results):
        ev, od = _unpack_core(np.asarray(r["y"]))
        sl = slice(c * RPC, (c + 1) * RPC)
        out[sl, 0, 0::2] = ev
        out[sl, 0, 1::2] = od
    # Row-boundary bits the device computed from junk halo cells;
    # recompute exactly in fp32 (reference add order, x[-1] = x[L] = 0).
    xf = x[:, 0, :]
    out[:, 0, 0] = (xf[:, 0] + xf[:, 1]) > 0.5
    out[:, 0, 2 * L - 2] = (xf[:, L - 2] + xf[:, L - 1]) > 0.5
    out[:, 0, 2 * L - 1] = xf[:, L - 1] > 0.5
    return out
